# revision 13
# baseline (speedup 1.0000x reference)
"""Multi-head self-attention (B=2, N=2048, D=1024, H=16, Dh=64) on 8 TRN2 NeuronCores.

Sharding: core c handles batch b = c // 4 and head group g = c % 4 (heads 4g..4g+3).
Tensor-parallel on heads for qkv/out_proj; data-parallel on batch. Each core
produces a partial [D, N] output (transposed); host sums the 4 head-group
partials per batch, transposes, and adds b_out.

All matmuls run in float32r (TF32-like PE mode, 1 cyc/row) with fp32 PSUM
accumulation; measured end-to-end relative error ~1e-4.
"""
import sys
import numpy as np

for _p in ("/opt/trn_rl_repo", "/root/.axon_site/_ro/trn_rl_repo"):
    if _p not in sys.path:
        sys.path.append(_p)

import concourse.bass as bass
import concourse.bacc as bacc
import concourse.tile as tile
from concourse import mybir
from concourse.bass_utils import run_bass_kernel_spmd

F32 = mybir.dt.float32
F32R = mybir.dt.float32r
EXP = mybir.ActivationFunctionType.Exp

B, S, D = 2, 2048, 1024
H, DH = 16, 64
HL = 4            # heads per core (local)
CQK = 512         # q+k channels per core (2*HL*DH)
CV = 256          # v channels per core (HL*DH)
ND = D // 128     # 8 d-tiles
NKT = S // 128    # 16 key tiles
NQC = S // 512    # 4 query chunks of 512


def build_kernel() -> "bass.Bass":
    nc = bacc.Bacc(None, target_bir_lowering=False, debug=False)

    xT = nc.dram_tensor("xT", [D, S], F32R, kind="ExternalInput")
    wqk = nc.dram_tensor("wqk", [D, CQK], F32R, kind="ExternalInput")
    bqk = nc.dram_tensor("bqk", [128, CQK // 128], F32, kind="ExternalInput")
    wv = nc.dram_tensor("wv", [D, CV], F32R, kind="ExternalInput")
    bvb = nc.dram_tensor("bvb", [128, CV], F32, kind="ExternalInput")
    wout = nc.dram_tensor("wout", [CV, D], F32R, kind="ExternalInput")
    outT = nc.dram_tensor("outT", [D, S], F32, kind="ExternalOutput")

    xT_r = xT.rearrange("(t p) s -> t p s", p=128)        # [8, 128, 2048]
    wqk_r = wqk.rearrange("(t p) c -> t p c", p=128)      # [8, 128, 512]
    wv_r = wv.rearrange("(t p) c -> t p c", p=128)        # [8, 128, 256]
    wout_r = wout.rearrange("(t p) n -> t p n", p=128)    # [2, 128, 1024]
    outT_r = outT.rearrange("(t p) s -> t p s", p=128)    # [8, 128, 2048]

    with tile.TileContext(nc) as tc:
        with tc.tile_pool(name="persist", bufs=1) as persist:
            # persistent tiles
            qkt_s = persist.tile([128, 4, S], F32R)          # [c-tile(q0 q1 k0 k1), seq]
            v_s = persist.tile([128, NKT, HL, DH + 1], F32R)  # per key-tile V + ones col
            at_s = persist.tile([128, 2, S], F32R)            # normalized attn out^T
            wout_s = persist.tile([128, 2, D], F32R)
            bqk_s = persist.tile([128, CQK // 128], F32)
            bvb_s = persist.tile([128, CV], F32)

            for t in range(2):
                nc.sync.dma_start(out=wout_s[:, t, :], in_=wout_r[t])
            nc.sync.dma_start(out=bqk_s[:], in_=bqk[:])
            nc.sync.dma_start(out=bvb_s[:], in_=bvb[:])
            nc.vector.memset(v_s[:, :, :, DH:DH + 1].bitcast(F32), 1.0)

            # ---------------- Phase A: projections ----------------
            pha_ctx = [tc.tile_pool(name="phA", bufs=1),
                       tc.tile_pool(name="phA_ps", bufs=1, space="PSUM"),
                       tc.tile_pool(name="phA_ps2", bufs=1, space="PSUM")]
            pha, pps, pps2 = [c.__enter__() for c in pha_ctx]
            xt_s = pha.tile([128, ND, S], F32R)
            wqk_s = pha.tile([128, ND, CQK], F32R)
            wv_s = pha.tile([128, ND, CV], F32R)
            for t in range(ND):
                nc.sync.dma_start(out=wqk_s[:, t, :], in_=wqk_r[t])
                nc.sync.dma_start(out=xt_s[:, t, :], in_=xT_r[t])
                nc.sync.dma_start(out=wv_s[:, t, :], in_=wv_r[t])

            # A1: q,k projection -> qkt_s (transposed layout, bias added, q pre-scaled)
            # order pair-0 tiles (m=0 q01, m=2 k01) first so phase B can start early
            for m in (0, 2, 1, 3):
                ps = [pps.tile([128, 512], F32, tag=f"qk{n}", name=f"psqk{n}")
                      for n in range(NQC)]
                for d in range(ND):
                    lhsT = wqk_s[:, d, m * 128:(m + 1) * 128]
                    for n in range(NQC):
                        nc.tensor.matmul(ps[n][:], lhsT,
                                         xt_s[:, d, n * 512:(n + 1) * 512],
                                         start=(d == 0), stop=(d == ND - 1))
                for n in range(NQC):
                    nc.vector.tensor_scalar_add(
                        qkt_s[:, m, n * 512:(n + 1) * 512], ps[n][:],
                        bqk_s[:, m:m + 1])

            # A2: v projection -> v_s (natural layout, bias via broadcast add).
            # 4 PSUM accumulation chains interleaved: a single chain serializes
            # the PE at ~610ns/mm; interleaved chains pipeline at ~250ns.
            for st0 in range(0, NKT, 4):
                psvs = [pps2.tile([128, CV], F32, tag=f"v{j}", name=f"psv{j}")
                        for j in range(4)]
                for d in range(ND):
                    for j in range(4):
                        st = st0 + j
                        nc.tensor.matmul(psvs[j][:],
                                         xt_s[:, d, st * 128:(st + 1) * 128],
                                         wv_s[:, d, :],
                                         start=(d == 0), stop=(d == ND - 1))
                for j in range(4):
                    st = st0 + j
                    nc.vector.tensor_tensor(
                        out=v_s[:, st, :, 0:DH],
                        in0=psvs[j][:].rearrange("p (h c) -> p h c", h=HL),
                        in1=bvb_s[:].rearrange("p (h c) -> p h c", h=HL),
                        op=mybir.AluOpType.add)

            for c in reversed(pha_ctx):
                c.__exit__(None, None, None)

            # ---------------- Phase B + C interleaved per query half ----------------
            stage_ctx = tc.tile_pool(name="stage", bufs=4)
            stage = stage_ctx.__enter__()
            bc_ctx = [tc.tile_pool(name="ptp", bufs=4),
                      tc.tile_pool(name="smallB", bufs=2),
                      tc.tile_pool(name="phB_s", bufs=1, space="PSUM"),
                      tc.tile_pool(name="phB_av", bufs=1, space="PSUM")]
            ptp, small, psb, psav = [c.__enter__() for c in bc_ctx]
            for qbh in range(2):    # query halves of 1024
                q0 = qbh * 1024
                for p in range(2):  # head pairs (2p, 2p+1)
                    qt = qkt_s[:, p, :]
                    kt = qkt_s[:, 2 + p, :]
                    pA = psav.tile([DH + 1, 1024], F32, tag="pA", name="pA")
                    pB = psav.tile([DH + 1, 1024], F32, tag="pB", name="pB")
                    for t in range(NKT):
                        sA = psb.tile([128, 1024], F32, tag="sA", name="sA")
                        sB = psb.tile([128, 1024], F32, tag="sB", name="sB")
                        for c in range(2):
                            qs = slice(q0 + c * 512, q0 + (c + 1) * 512)
                            ls = slice(c * 512, (c + 1) * 512)
                            nc.tensor.matmul(sA[:, ls],
                                             kt[0:64, t * 128:(t + 1) * 128],
                                             qt[0:64, qs], start=True, stop=True,
                                             tile_position=(0, 0))
                            nc.tensor.matmul(sB[:, ls],
                                             kt[64:128, t * 128:(t + 1) * 128],
                                             qt[64:128, qs], start=True, stop=True,
                                             tile_position=(64, 0))
                        ptA = ptp.tile([128, 1024], F32R, tag="ptA", name="ptA")
                        ptB = ptp.tile([128, 1024], F32R, tag="ptB", name="ptB")
                        nc.scalar.activation(ptA[:], sA[:], EXP)
                        nc.scalar.activation(ptB[:], sB[:], EXP)
                        for c in range(2):
                            ls = slice(c * 512, (c + 1) * 512)
                            nc.tensor.matmul(pA[:, ls], v_s[:, t, 2 * p, :],
                                             ptA[:, ls],
                                             start=(t == 0), stop=(t == NKT - 1))
                            nc.tensor.matmul(pB[:, ls], v_s[:, t, 2 * p + 1, :],
                                             ptB[:, ls],
                                             start=(t == 0), stop=(t == NKT - 1))
                    # normalize by softmax denominator (row DH of each psum).
                    # reciprocal on [1,1024] is a 6.5us single-lane op; reshape
                    # across partitions via DMA to make it a wide op instead.
                    for loc, pX in ((0, pA), (1, pB)):
                        hh = 2 * p + loc
                        # evict whole psum (releases banks early), then build
                        # 1/denom without any single-lane DVE op: DMA-reshape
                        # the denom row across 64 partitions, wide reciprocal,
                        # DMA back, partition-broadcast.
                        raw = small.tile([DH + 1, 1024], F32, tag="raw", name="raw")
                        nc.vector.tensor_copy(out=raw[:], in_=pX[:])
                        dn = small.tile([64, 16], F32, tag="dn", name="dn")
                        nc.sync.dma_start(out=dn[:], in_=raw[DH:DH + 1, :])
                        rr = small.tile([64, 16], F32, tag="rr", name="rr")
                        nc.vector.reciprocal(rr[:], dn[:])
                        r = small.tile([1, 1024], F32, tag="r", name="r")
                        nc.sync.dma_start(out=r[:], in_=rr[:])
                        rb = small.tile([64, 1024], F32, tag="rb", name="rb")
                        nc.gpsimd.partition_broadcast(rb[:], r[:])
                        if loc == 0:
                            nc.vector.tensor_tensor(
                                out=at_s[0:64, hh // 2, q0:q0 + 1024],
                                in0=raw[0:DH, :], in1=rb[:],
                                op=mybir.AluOpType.mult)
                        else:
                            # DVE lanes cannot shift partitions; bounce via DMA
                            tmp = small.tile([64, 1024], F32R, tag="tmp", name="tmp")
                            nc.vector.tensor_tensor(
                                out=tmp[:], in0=raw[0:DH, :], in1=rb[:],
                                op=mybir.AluOpType.mult)
                            nc.sync.dma_start(
                                out=at_s[64:128, hh // 2, q0:q0 + 1024],
                                in_=tmp[:])

            for c in reversed(bc_ctx):
                c.__exit__(None, None, None)

            # ---------------- Phase C: out^T = wout^T @ at ----------------
            c_ctx = [tc.tile_pool(name="phC_ps", bufs=1, space="PSUM")]
            psc = c_ctx[0].__enter__()
            for nt in range(ND):
                pos = [psc.tile([128, 512], F32, tag=f"o{qc}", name=f"po{qc}")
                       for qc in range(NQC)]
                for ct in range(2):
                    lhsT = wout_s[:, ct, nt * 128:(nt + 1) * 128]
                    for qc in range(NQC):
                        nc.tensor.matmul(pos[qc][:], lhsT,
                                         at_s[:, ct, qc * 512:(qc + 1) * 512],
                                         start=(ct == 0), stop=(ct == 1))
                for qc in range(NQC):
                    qg = slice(qc * 512, (qc + 1) * 512)
                    o = stage.tile([128, 512], F32, tag="o", name="o")
                    nc.vector.tensor_copy(out=o[:], in_=pos[qc][:])
                    nc.sync.dma_start(out=outT_r[nt][:, qg], in_=o[:])
            c_ctx[0].__exit__(None, None, None)
            stage_ctx.__exit__(None, None, None)
    nc.compile()
    return nc


def shard_inputs(x, W_qkv, b_qkv, W_out, b_out=None):
    """Build the 8 per-core input maps. Core c: batch c//4, head group c%4."""
    in_maps = []
    scale = 1.0 / np.sqrt(np.float32(DH))
    for c in range(8):
        b, g = divmod(c, 4)
        cs = slice(g * 256, g * 256 + 256)
        xTc = np.ascontiguousarray(x[b].T)                       # [D, S]
        wq = W_qkv[:, 0:D][:, cs] * scale                        # [D, 256]
        wk = W_qkv[:, D:2 * D][:, cs]
        wqk = np.ascontiguousarray(np.concatenate([wq, wk], axis=1))  # [D, 512]
        bq = b_qkv[0:D][cs] * scale
        bk = b_qkv[D:2 * D][cs]
        bqk = np.concatenate([bq, bk]).reshape(CQK // 128, 128).T     # [128, 4]
        bqk = np.ascontiguousarray(bqk)
        wv = np.ascontiguousarray(W_qkv[:, 2 * D:3 * D][:, cs])       # [D, 256]
        bvb = np.ascontiguousarray(
            np.broadcast_to(b_qkv[2 * D:3 * D][cs], (128, CV)))       # [128, 256]
        woutc = np.ascontiguousarray(W_out[cs, :])                    # [256, D]
        in_maps.append({
            "xT": xTc.astype(np.float32),
            "wqk": wqk.astype(np.float32),
            "bqk": bqk.astype(np.float32),
            "wv": wv.astype(np.float32),
            "bvb": bvb.astype(np.float32),
            "wout": woutc.astype(np.float32),
        })
    return in_maps


_NC_CACHE = []


def _get_nc():
    if not _NC_CACHE:
        _NC_CACHE.append(build_kernel())
    return _NC_CACHE[0]


def run_sharded(in_maps, **kwargs):
    nc = _get_nc()
    return run_bass_kernel_spmd(nc, in_maps, core_ids=list(range(8)), **kwargs)


def gather_output(results, b_out):
    out = np.empty((B, S, D), dtype=np.float32)
    for b in range(B):
        acc = results[4 * b]["outT"].astype(np.float32).copy()
        for g in range(1, 4):
            acc += results[4 * b + g]["outT"]
        out[b] = acc.T + b_out[None, :]
    return out


def kernel(x, W_qkv, b_qkv, W_out, b_out):
    x = np.asarray(x, dtype=np.float32)
    W_qkv = np.asarray(W_qkv, dtype=np.float32)
    b_qkv = np.asarray(b_qkv, dtype=np.float32)
    W_out = np.asarray(W_out, dtype=np.float32)
    b_out = np.asarray(b_out, dtype=np.float32)
    in_maps = shard_inputs(x=x, W_qkv=W_qkv, b_qkv=b_qkv, W_out=W_out, b_out=b_out)
    res = run_sharded(in_maps)
    return gather_output(res.results, b_out)


# revision 15
# speedup vs baseline: 1.1105x; 1.1105x over previous
"""Multi-head self-attention (B=2, N=2048, D=1024, H=16, Dh=64) on 8 TRN2 NeuronCores.

Sharding: core c handles batch b = c // 4 and head group g = c % 4 (heads 4g..4g+3).
Tensor-parallel on heads for qkv/out_proj; data-parallel on batch. Each core
produces a partial [D, N] output (transposed); host sums the 4 head-group
partials per batch, transposes, and adds b_out.

All matmuls run in float32r (TF32-like PE mode, 1 cyc/row) with fp32 PSUM
accumulation; measured end-to-end relative error ~1e-4.
"""
import sys
import numpy as np

for _p in ("/opt/trn_rl_repo", "/root/.axon_site/_ro/trn_rl_repo"):
    if _p not in sys.path:
        sys.path.append(_p)

import concourse.bass as bass
import concourse.bacc as bacc
import concourse.tile as tile
from concourse import mybir
from concourse.bass_utils import run_bass_kernel_spmd

F32 = mybir.dt.float32
F32R = mybir.dt.float32r
EXP = mybir.ActivationFunctionType.Exp

B, S, D = 2, 2048, 1024
H, DH = 16, 64
HL = 4            # heads per core (local)
CQK = 512         # q+k channels per core (2*HL*DH)
CV = 256          # v channels per core (HL*DH)
ND = D // 128     # 8 d-tiles
NKT = S // 128    # 16 key tiles
NQC = S // 512    # 4 query chunks of 512


def build_kernel() -> "bass.Bass":
    nc = bacc.Bacc(None, target_bir_lowering=False, debug=False)

    xT = nc.dram_tensor("xT", [D, S], F32R, kind="ExternalInput")
    wqk = nc.dram_tensor("wqk", [D, CQK], F32R, kind="ExternalInput")
    bqk = nc.dram_tensor("bqk", [128, CQK // 128], F32, kind="ExternalInput")
    wv = nc.dram_tensor("wv", [D, CV], F32R, kind="ExternalInput")
    bvb = nc.dram_tensor("bvb", [128, CV], F32, kind="ExternalInput")
    wout = nc.dram_tensor("wout", [CV, D], F32R, kind="ExternalInput")
    outT = nc.dram_tensor("outT", [D, S], F32, kind="ExternalOutput")

    xT_r = xT.rearrange("(t p) s -> t p s", p=128)        # [8, 128, 2048]
    wqk_r = wqk.rearrange("(t p) c -> t p c", p=128)      # [8, 128, 512]
    wv_r = wv.rearrange("(t p) c -> t p c", p=128)        # [8, 128, 256]
    wout_r = wout.rearrange("(t p) n -> t p n", p=128)    # [2, 128, 1024]
    outT_r = outT.rearrange("(t p) s -> t p s", p=128)    # [8, 128, 2048]

    with tile.TileContext(nc) as tc:
        with tc.tile_pool(name="persist", bufs=1) as persist:
            # persistent tiles (slim: at_s allocated later, after phase-A SBUF frees)
            qkt_s = persist.tile([128, 4, S], F32R)          # [c-tile(q0 q1 k0 k1), seq]
            v_s = persist.tile([128, NKT, HL, DH + 1], F32R)  # per key-tile V + ones col
            wout_s = persist.tile([128, 2, D], F32R)
            bqk_s = persist.tile([128, CQK // 128], F32)
            bvb_s = persist.tile([128, CV], F32)

            for t in range(2):
                nc.sync.dma_start(out=wout_s[:, t, :], in_=wout_r[t])
            nc.sync.dma_start(out=bqk_s[:], in_=bqk[:])
            nc.sync.dma_start(out=bvb_s[:], in_=bvb[:])
            nc.vector.memset(v_s[:, :, :, DH:DH + 1].bitcast(F32), 1.0)

            # ---------------- Phase A: projections ----------------
            # PSUM bank choreography: A1 first half (ppsA) -> release banks to
            # phase B's S^T psums (psb) so B starts mid-A1 and the PE never
            # dips at the A->B boundary (a dip re-throttles the HAM clock).
            pha_ctx = tc.tile_pool(name="phA", bufs=1)
            pha = pha_ctx.__enter__()
            xt_s = pha.tile([128, ND, S], F32R)
            wqk_s = pha.tile([128, ND, CQK], F32R)
            wv_s = pha.tile([128, ND, CV], F32R)
            for t in range(ND):
                nc.sync.dma_start(out=wqk_s[:, t, :], in_=wqk_r[t])
                nc.sync.dma_start(out=xt_s[:, t, :], in_=xT_r[t])
                nc.sync.dma_start(out=wv_s[:, t, :], in_=wv_r[t])

            def a1_pass(pool, m):
                ps = [pool.tile([128, 512], F32, tag=f"qk{n}", name=f"psqk{n}")
                      for n in range(NQC)]
                for d in range(ND):
                    lhsT = wqk_s[:, d, m * 128:(m + 1) * 128]
                    for n in range(NQC):
                        nc.tensor.matmul(ps[n][:], lhsT,
                                         xt_s[:, d, n * 512:(n + 1) * 512],
                                         start=(d == 0), stop=(d == ND - 1))
                for n in range(NQC):
                    nc.vector.tensor_scalar_add(
                        qkt_s[:, m, n * 512:(n + 1) * 512], ps[n][:],
                        bqk_s[:, m:m + 1])

            ppsA_ctx = tc.tile_pool(name="ppsA", bufs=1, space="PSUM")
            ppsA = ppsA_ctx.__enter__()
            for m in (0, 2):      # pair-0 q and k tiles first
                a1_pass(ppsA, m)
            ppsA_ctx.__exit__(None, None, None)

            # phase-B pools open now: psb reuses ppsA's banks; ptp/smallB get
            # SBUF disjoint from phase-A tiles so early S^T/exp can run.
            psb_ctx = tc.tile_pool(name="phB_s", bufs=1, space="PSUM", side="right")
            psb = psb_ctx.__enter__()
            ptp_ctx = tc.tile_pool(name="ptp", bufs=2, side="right")
            ptp = ptp_ctx.__enter__()
            small_ctx = tc.tile_pool(name="smallB", bufs=1, side="right")
            small = small_ctx.__enter__()

            ppsB_ctx = tc.tile_pool(name="ppsB", bufs=1, space="PSUM")
            ppsB = ppsB_ctx.__enter__()
            for m in (1, 3):
                a1_pass(ppsB, m)
            ppsB_ctx.__exit__(None, None, None)

            # A2: v projection -> v_s (natural layout, bias via broadcast add).
            # 4 PSUM accumulation chains interleaved: a single chain serializes
            # the PE at ~610ns/mm; interleaved chains pipeline at ~250ns.
            pps2_ctx = tc.tile_pool(name="phA_ps2", bufs=1, space="PSUM")
            pps2 = pps2_ctx.__enter__()
            for st0 in range(0, NKT, 4):
                psvs = [pps2.tile([128, CV], F32, tag=f"v{j}", name=f"psv{j}")
                        for j in range(4)]
                for d in range(ND):
                    for j in range(4):
                        st = st0 + j
                        nc.tensor.matmul(psvs[j][:],
                                         xt_s[:, d, st * 128:(st + 1) * 128],
                                         wv_s[:, d, :],
                                         start=(d == 0), stop=(d == ND - 1))
                for j in range(4):
                    st = st0 + j
                    nc.vector.tensor_tensor(
                        out=v_s[:, st, :, 0:DH],
                        in0=psvs[j][:].rearrange("p (h c) -> p h c", h=HL),
                        in1=bvb_s[:].rearrange("p (h c) -> p h c", h=HL),
                        op=mybir.AluOpType.add)
            pps2_ctx.__exit__(None, None, None)
            pha_ctx.__exit__(None, None, None)

            # ---------------- Phase B + C ----------------
            late_ctx = [tc.tile_pool(name="atp", bufs=1, side="right"),
                        tc.tile_pool(name="stage", bufs=4, side="right"),
                        tc.tile_pool(name="phB_av", bufs=1, space="PSUM", side="right")]
            atp, stage, psav = [c.__enter__() for c in late_ctx]
            at_s = atp.tile([128, 2, S], F32R)               # normalized attn out^T
            bc_ctx = [psb_ctx, ptp_ctx, small_ctx] + late_ctx
            for qbh in range(2):    # query halves of 1024
                q0 = qbh * 1024
                for p in range(2):  # head pairs (2p, 2p+1)
                    qt = qkt_s[:, p, :]
                    kt = qkt_s[:, 2 + p, :]
                    pA = psav.tile([DH + 1, 1024], F32, tag="pA", name="pA")
                    pB = psav.tile([DH + 1, 1024], F32, tag="pB", name="pB")
                    for t in range(NKT):
                        sA = psb.tile([128, 1024], F32, tag="sA", name="sA")
                        sB = psb.tile([128, 1024], F32, tag="sB", name="sB")
                        for c in range(2):
                            qs = slice(q0 + c * 512, q0 + (c + 1) * 512)
                            ls = slice(c * 512, (c + 1) * 512)
                            nc.tensor.matmul(sA[:, ls],
                                             kt[0:64, t * 128:(t + 1) * 128],
                                             qt[0:64, qs], start=True, stop=True,
                                             tile_position=(0, 0))
                            nc.tensor.matmul(sB[:, ls],
                                             kt[64:128, t * 128:(t + 1) * 128],
                                             qt[64:128, qs], start=True, stop=True,
                                             tile_position=(64, 0))
                        ptA = ptp.tile([128, 1024], F32R, tag="ptA", name="ptA")
                        ptB = ptp.tile([128, 1024], F32R, tag="ptB", name="ptB")
                        nc.scalar.activation(ptA[:], sA[:], EXP)
                        nc.scalar.activation(ptB[:], sB[:], EXP)
                        for c in range(2):
                            ls = slice(c * 512, (c + 1) * 512)
                            nc.tensor.matmul(pA[:, ls], v_s[:, t, 2 * p, :],
                                             ptA[:, ls],
                                             start=(t == 0), stop=(t == NKT - 1))
                            nc.tensor.matmul(pB[:, ls], v_s[:, t, 2 * p + 1, :],
                                             ptB[:, ls],
                                             start=(t == 0), stop=(t == NKT - 1))
                    # normalize by softmax denominator (row DH of each psum).
                    # reciprocal on [1,1024] is a 6.5us single-lane op; reshape
                    # across partitions via DMA to make it a wide op instead.
                    for loc, pX in ((0, pA), (1, pB)):
                        hh = 2 * p + loc
                        # evict whole psum (releases banks early), then build
                        # 1/denom without any single-lane DVE op: DMA-reshape
                        # the denom row across 64 partitions, wide reciprocal,
                        # DMA back, partition-broadcast.
                        raw = small.tile([DH + 1, 1024], F32, tag="raw", name="raw")
                        nc.vector.tensor_copy(out=raw[:], in_=pX[:])
                        dn = small.tile([64, 16], F32, tag="dn", name="dn")
                        nc.sync.dma_start(out=dn[:], in_=raw[DH:DH + 1, :])
                        rr = small.tile([64, 16], F32, tag="rr", name="rr")
                        nc.vector.reciprocal(rr[:], dn[:])
                        r = small.tile([1, 1024], F32, tag="r", name="r")
                        nc.sync.dma_start(out=r[:], in_=rr[:])
                        rb = small.tile([64, 1024], F32, tag="rb", name="rb")
                        nc.gpsimd.partition_broadcast(rb[:], r[:])
                        if loc == 0:
                            nc.vector.tensor_tensor(
                                out=at_s[0:64, hh // 2, q0:q0 + 1024],
                                in0=raw[0:DH, :], in1=rb[:],
                                op=mybir.AluOpType.mult)
                        else:
                            # DVE lanes cannot shift partitions; bounce via DMA
                            tmp = small.tile([64, 1024], F32R, tag="tmp", name="tmp")
                            nc.vector.tensor_tensor(
                                out=tmp[:], in0=raw[0:DH, :], in1=rb[:],
                                op=mybir.AluOpType.mult)
                            nc.sync.dma_start(
                                out=at_s[64:128, hh // 2, q0:q0 + 1024],
                                in_=tmp[:])

            late_ctx[2].__exit__(None, None, None)   # psav (right-stack top)

            # ---------------- Phase C: out^T = wout^T @ at ----------------
            c_ctx = [tc.tile_pool(name="phC_ps", bufs=1, space="PSUM", side="right")]
            psc = c_ctx[0].__enter__()
            for nt in range(ND):
                pos = [psc.tile([128, 512], F32, tag=f"o{qc}", name=f"po{qc}")
                       for qc in range(NQC)]
                for ct in range(2):
                    lhsT = wout_s[:, ct, nt * 128:(nt + 1) * 128]
                    for qc in range(NQC):
                        nc.tensor.matmul(pos[qc][:], lhsT,
                                         at_s[:, ct, qc * 512:(qc + 1) * 512],
                                         start=(ct == 0), stop=(ct == 1))
                for qc in range(NQC):
                    qg = slice(qc * 512, (qc + 1) * 512)
                    o = stage.tile([128, 512], F32, tag="o", name="o")
                    nc.vector.tensor_copy(out=o[:], in_=pos[qc][:])
                    nc.sync.dma_start(out=outT_r[nt][:, qg], in_=o[:])
            c_ctx[0].__exit__(None, None, None)
            late_ctx[1].__exit__(None, None, None)   # stage
            late_ctx[0].__exit__(None, None, None)   # atp
            small_ctx.__exit__(None, None, None)
            ptp_ctx.__exit__(None, None, None)
            psb_ctx.__exit__(None, None, None)
    nc.compile()
    return nc


def shard_inputs(x, W_qkv, b_qkv, W_out, b_out=None):
    """Build the 8 per-core input maps. Core c: batch c//4, head group c%4."""
    in_maps = []
    scale = 1.0 / np.sqrt(np.float32(DH))
    for c in range(8):
        b, g = divmod(c, 4)
        cs = slice(g * 256, g * 256 + 256)
        xTc = np.ascontiguousarray(x[b].T)                       # [D, S]
        wq = W_qkv[:, 0:D][:, cs] * scale                        # [D, 256]
        wk = W_qkv[:, D:2 * D][:, cs]
        wqk = np.ascontiguousarray(np.concatenate([wq, wk], axis=1))  # [D, 512]
        bq = b_qkv[0:D][cs] * scale
        bk = b_qkv[D:2 * D][cs]
        bqk = np.concatenate([bq, bk]).reshape(CQK // 128, 128).T     # [128, 4]
        bqk = np.ascontiguousarray(bqk)
        wv = np.ascontiguousarray(W_qkv[:, 2 * D:3 * D][:, cs])       # [D, 256]
        bvb = np.ascontiguousarray(
            np.broadcast_to(b_qkv[2 * D:3 * D][cs], (128, CV)))       # [128, 256]
        woutc = np.ascontiguousarray(W_out[cs, :])                    # [256, D]
        in_maps.append({
            "xT": xTc.astype(np.float32),
            "wqk": wqk.astype(np.float32),
            "bqk": bqk.astype(np.float32),
            "wv": wv.astype(np.float32),
            "bvb": bvb.astype(np.float32),
            "wout": woutc.astype(np.float32),
        })
    return in_maps


_NC_CACHE = []


def _get_nc():
    if not _NC_CACHE:
        _NC_CACHE.append(build_kernel())
    return _NC_CACHE[0]


def run_sharded(in_maps, **kwargs):
    nc = _get_nc()
    return run_bass_kernel_spmd(nc, in_maps, core_ids=list(range(8)), **kwargs)


def gather_output(results, b_out):
    out = np.empty((B, S, D), dtype=np.float32)
    for b in range(B):
        acc = results[4 * b]["outT"].astype(np.float32).copy()
        for g in range(1, 4):
            acc += results[4 * b + g]["outT"]
        out[b] = acc.T + b_out[None, :]
    return out


def kernel(x, W_qkv, b_qkv, W_out, b_out):
    x = np.asarray(x, dtype=np.float32)
    W_qkv = np.asarray(W_qkv, dtype=np.float32)
    b_qkv = np.asarray(b_qkv, dtype=np.float32)
    W_out = np.asarray(W_out, dtype=np.float32)
    b_out = np.asarray(b_out, dtype=np.float32)
    in_maps = shard_inputs(x=x, W_qkv=W_qkv, b_qkv=b_qkv, W_out=W_out, b_out=b_out)
    res = run_sharded(in_maps)
    return gather_output(res.results, b_out)
